# revision 14
# baseline (speedup 1.0000x reference)
# BinarizeLinear on 8 Trainium2 NeuronCores.
#
# reference: out = binarize(x) @ binarize(weight).T + bias
#   x      [16384, 2048] f32
#   weight [2048, 2048]  f32
#   bias   [2048]        f32
#   out    [16384, 2048] f32
#
# Strategy (data-parallel over rows of x, weight/bias replicated):
#   - Each of the 8 cores gets a 2048-row shard of x.
#   - Host uploads x-shard and weight TRANSPOSED (K on the leading axis) so
#     the contraction dim lands on SBUF partitions with a natural contiguous
#     DMA.  Uploads are fp8e4m3 with magnitudes clipped into fp8 range and
#     zeros encoded as tiny negatives: a lossless SIGN encoding, which is the
#     only thing binarize consumes (reference maps 0 -> -1, hence -0 style
#     encoding for zeros).
#   - Device binarizes both operands to exactly +-1.0 with a single
#     tensor_scalar pass over a uint8 bitcast: (v & 0x80) | 0x38.
#   - out.T[n, m] = sum_k wbT[k, n] * xbT[k, m] accumulates in PSUM with
#     DoubleRow fp8 matmuls (2 MACs/cell/cycle, contraction 256 per MM).
#   - ScalarE evacuates PSUM with a fused per-partition bias add
#     (activation Identity, bias = bias[n] column), giving out.T + bias.
#   - Host transposes each core's out.T shard back and stacks.

import sys

import numpy as np

try:
    import concourse  # noqa: F401
except ImportError:
    sys.path.insert(0, "/opt/trn_rl_repo")

import ml_dtypes
from contextlib import ExitStack

import concourse.bass as bass
import concourse.mybir as mybir
import concourse.tile as tile
from concourse import bacc
from concourse.bass_utils import run_bass_kernel_spmd

NCORES = 8
K = 2048          # contraction dim (in_features)
NF = 2048         # out features
MTOT = 16384      # rows of x
MS = MTOT // NCORES  # rows per core
P = 128           # partitions
MC = 512          # moving free-dim chunk (one PSUM bank of f32)
KT2 = K // (2 * P)   # 8 double-k-tiles (DoubleRow contracts 256/MM)
NT = NF // P      # 16 n-tiles
MT = MS // MC     # 4 m-chunks

F32 = mybir.dt.float32
FP8 = mybir.dt.float8e4
U8 = mybir.dt.uint8


def build_nc(debug=False):
    nc = bacc.Bacc(
        "TRN2", target_bir_lowering=False, debug=debug, num_devices=NCORES
    )
    # Inputs arrive pre-tiled from the host: [t, h, p, j, c] =
    # transposed_tensor[(2t+j)*128 + p, h*CH + c], so each (t, h) chunk is
    # one fully contiguous [128, 2, CH] DMA.  x uses halves (CH=1024, both
    # needed by every group); w uses quarters (CH=512) so the ramp only
    # streams the columns the current n-groups consume.
    XCH = NF // 2
    WCH = NF // 4
    xT = nc.dram_tensor(
        "xT", [K // (2 * P), 2, P, 2, XCH], FP8, kind="ExternalInput"
    ).ap()
    wT = nc.dram_tensor(
        "wT", [K // (2 * P), 4, P, 2, WCH], FP8, kind="ExternalInput"
    ).ap()
    bias = nc.dram_tensor("bias", [NF], F32, kind="ExternalInput").ap()
    outT = nc.dram_tensor("outT", [NF, MS], F32, kind="ExternalOutput").ap()

    U16 = mybir.dt.uint16
    NG = 2  # n-tiles per group; NG*MT psum banks live at once

    with tile.TileContext(nc) as tc:
        with ExitStack() as ctx:
            const = ctx.enter_context(tc.tile_pool(name="const", bufs=1))
            res = ctx.enter_context(tc.tile_pool(name="res", bufs=1))
            psum = ctx.enter_context(
                tc.tile_pool(name="ps", bufs=1, space=bass.MemorySpace.PSUM)
            )
            outp = ctx.enter_context(tc.tile_pool(name="out", bufs=3))

            # bias[n] laid out [128, 16]: column t holds bias[t*128:(t+1)*128]
            bias_t = const.tile([P, NT], F32)
            nc.gpsimd.dma_start(
                out=bias_t[:], in_=bias.rearrange("(t p) -> p t", p=P)
            )

            def load_bin(dram, name, t, h, width, dma_eng):
                # DoubleRow operand chunk [128, 2, width] fp8, contiguous DMA.
                # Binarize in place: keep sign bit, force the rest to 1.0.
                # The pass runs on a uint16 view (two fp8 per ALU element).
                tl = res.tile([P, 2, width], FP8, tag=f"{name}{t}_{h}")
                dma_eng.dma_start(out=tl[:], in_=dram[t, h])
                nc.vector.tensor_scalar(
                    tl[:].bitcast(U16),
                    tl[:].bitcast(U16),
                    0x8080,
                    0x3838,
                    mybir.AluOpType.bitwise_and,
                    mybir.AluOpType.bitwise_or,
                )
                return tl

            # x: both halves feed every group -> interleave by strip.
            # w quarter q only feeds n-tiles 4q..4q+3 (groups 2q, 2q+1) ->
            # stream quarter-major so early HBM bandwidth unblocks group 0.
            wb = [[None] * 4 for _ in range(KT2)]
            xb = []
            for t in range(KT2):
                wb[t][0] = load_bin(wT, "w", t, 0, WCH, nc.sync)
                xb.append(
                    [
                        load_bin(xT, "x", t, 0, XCH, nc.scalar),
                        load_bin(xT, "x", t, 1, XCH, nc.scalar),
                    ]
                )
            for q in range(1, 4):
                for t in range(KT2):
                    wb[t][q] = load_bin(wT, "w", t, q, WCH, nc.sync)

            NPQ = WCH // P   # n-tiles per w quarter
            MCH = XCH // MC  # m-chunks per x half

            def w_slice(t, n):
                return wb[t][n // NPQ][
                    :, :, (n % NPQ) * P : (n % NPQ + 1) * P
                ]

            def x_slice(t, mc):
                return xb[t][mc // MCH][
                    :, :, (mc % MCH) * MC : (mc % MCH + 1) * MC
                ]

            NGRP = NT // NG
            for g in range(NGRP):
                pss = [
                    [
                        psum.tile(
                            [P, MC], F32, tag=f"ps{i}_{mc}", name=f"ps_{g}_{i}_{mc}"
                        )
                        for mc in range(MT)
                    ]
                    for i in range(NG)
                ]
                ots = [
                    outp.tile([P, MS], F32, tag=f"o{i}", name=f"o_{g}_{i}")
                    for i in range(NG)
                ]

                def evacuate(i, mc):
                    n = g * NG + i
                    nc.scalar.activation(
                        ots[i][:, mc * MC : (mc + 1) * MC],
                        pss[i][mc][:],
                        mybir.ActivationFunctionType.Identity,
                        bias=bias_t[:, n : n + 1],
                    )
                    nc.sync.dma_start(
                        out=outT[n * P : (n + 1) * P, mc * MC : (mc + 1) * MC],
                        in_=ots[i][:, mc * MC : (mc + 1) * MC],
                    )

                if g < NGRP - 1:
                    # k-tile outer: consume input strips as they stream in.
                    for t in range(KT2):
                        for i in range(NG):
                            for mc in range(MT):
                                nc.tensor.matmul(
                                    pss[i][mc][:],
                                    w_slice(t, g * NG + i),
                                    x_slice(t, mc),
                                    start=(t == 0),
                                    stop=(t == KT2 - 1),
                                    perf_mode=mybir.MatmulPerfMode.DoubleRow,
                                )
                    for i in range(NG):
                        for mc in range(MT):
                            evacuate(i, mc)
                else:
                    # Last group: bank-major so evacuation and output DMA of
                    # bank b overlap the matmuls of bank b+1 (shrinks the
                    # kernel tail to one bank's epilogue).
                    for i in range(NG):
                        for mc in range(MT):
                            for t in range(KT2):
                                nc.tensor.matmul(
                                    pss[i][mc][:],
                                    w_slice(t, g * NG + i),
                                    x_slice(t, mc),
                                    start=(t == 0),
                                    stop=(t == KT2 - 1),
                                    perf_mode=mybir.MatmulPerfMode.DoubleRow,
                                )
                            evacuate(i, mc)

    nc.compile()
    return nc


_NC = None


def _get_nc():
    global _NC
    if _NC is None:
        _NC = build_nc()
    return _NC


def _to_fp8_T(a, nchunks):
    # Transposed fp8 copy preserving the SIGN of every element exactly
    # (magnitudes are irrelevant downstream -- the device binarizes).
    # Magnitudes are clipped into e4m3 range so the cast can't flush to
    # zero or overflow, and exact zeros are encoded as tiny NEGATIVES
    # because reference binarize maps 0 -> -1.  The result is pre-tiled to
    # [t, chunk, p, j, c] so each device chunk is one contiguous DMA.
    at = a.T
    mag = np.clip(np.abs(at), 0.002, 240.0)
    enc = np.where(at > 0, mag, -mag).astype(ml_dtypes.float8_e4m3fn)
    kk, cols = enc.shape
    tiled = enc.reshape(
        kk // (2 * P), 2, P, nchunks, cols // nchunks
    ).transpose(0, 3, 2, 1, 4)
    return np.ascontiguousarray(tiled)


def make_in_maps(x, weight, bias):
    x = np.asarray(x, dtype=np.float32)
    weight = np.asarray(weight, dtype=np.float32)
    bias = np.asarray(bias, dtype=np.float32)
    wTb = _to_fp8_T(weight, 4)
    in_maps = []
    for i in range(NCORES):
        xTb = _to_fp8_T(x[i * MS : (i + 1) * MS, :], 2)
        in_maps.append({"xT": xTb, "wT": wTb, "bias": bias})
    return in_maps


def assemble_out(results):
    out = np.empty((MTOT, NF), dtype=np.float32)
    for i in range(NCORES):
        out[i * MS : (i + 1) * MS, :] = results[i]["outT"].T
    return out


def run(x, weight, bias, trace=False, **kwargs):
    nc = _get_nc()
    in_maps = make_in_maps(x, weight, bias)
    res = run_bass_kernel_spmd(
        nc, in_maps, list(range(NCORES)), trace=trace, **kwargs
    )
    return assemble_out(res.results), res


def kernel(x, weight, bias):
    out, _ = run(x, weight, bias)
    return out


# revision 18
# speedup vs baseline: 1.0103x; 1.0103x over previous
# BinarizeLinear on 8 Trainium2 NeuronCores.
#
# reference: out = binarize(x) @ binarize(weight).T + bias
#   x      [16384, 2048] f32
#   weight [2048, 2048]  f32
#   bias   [2048]        f32
#   out    [16384, 2048] f32
#
# Strategy (data-parallel over rows of x, weight/bias replicated):
#   - Each of the 8 cores gets a 2048-row shard of x.
#   - Host uploads x-shard and weight TRANSPOSED (K on the leading axis) so
#     the contraction dim lands on SBUF partitions with a natural contiguous
#     DMA.  Uploads are fp8e4m3 with magnitudes clipped into fp8 range and
#     zeros encoded as tiny negatives: a lossless SIGN encoding, which is the
#     only thing binarize consumes (reference maps 0 -> -1, hence -0 style
#     encoding for zeros).
#   - Device binarizes both operands to exactly +-1.0 with a single
#     tensor_scalar pass over a uint16 bitcast (two fp8 lanes per element):
#     (v & 0x8080) | 0x3838.
#   - out.T[n, m] = sum_k wbT[k, n] * xbT[k, m] accumulates in PSUM with
#     DoubleRow fp8 matmuls (2 MACs/cell/cycle, contraction 256 per MM).
#   - ScalarE evacuates PSUM with a fused per-partition bias add
#     (activation Identity, bias = bias[n] column), giving out.T + bias.
#   - Host transposes each core's out.T shard back and stacks.

import sys

import numpy as np

try:
    import concourse  # noqa: F401
except ImportError:
    sys.path.insert(0, "/opt/trn_rl_repo")

import ml_dtypes
from contextlib import ExitStack

import concourse.bass as bass
import concourse.mybir as mybir
import concourse.tile as tile
from concourse import bacc
from concourse.bass_utils import run_bass_kernel_spmd

NCORES = 8
K = 2048          # contraction dim (in_features)
NF = 2048         # out features
MTOT = 16384      # rows of x
MS = MTOT // NCORES  # rows per core
P = 128           # partitions
MC = 512          # moving free-dim chunk (one PSUM bank of f32)
KT2 = K // (2 * P)   # 8 double-k-tiles (DoubleRow contracts 256/MM)
NT = NF // P      # 16 n-tiles
MT = MS // MC     # 4 m-chunks

F32 = mybir.dt.float32
FP8 = mybir.dt.float8e4
U8 = mybir.dt.uint8


def build_nc(debug=False):
    nc = bacc.Bacc(
        "TRN2", target_bir_lowering=False, debug=debug, num_devices=NCORES
    )
    # Inputs arrive pre-tiled from the host: [t, h, p, j, c] =
    # transposed_tensor[(2t+j)*128 + p, h*CH + c], so each (t, h) chunk is
    # one fully contiguous [128, 2, CH] DMA (2KB per partition).
    XCH = NF // 2
    WCH = NF // 2
    xT = nc.dram_tensor(
        "xT", [K // (2 * P), 2, P, 2, XCH], FP8, kind="ExternalInput"
    ).ap()
    wT = nc.dram_tensor(
        "wT", [K // (2 * P), 2, P, 2, WCH], FP8, kind="ExternalInput"
    ).ap()
    bias = nc.dram_tensor("bias", [NF], F32, kind="ExternalInput").ap()
    outT = nc.dram_tensor("outT", [NF, MS], F32, kind="ExternalOutput").ap()

    U16 = mybir.dt.uint16
    NG = 2  # n-tiles per group; NG*MT psum banks live at once

    with tile.TileContext(nc) as tc:
        with ExitStack() as ctx:
            const = ctx.enter_context(tc.tile_pool(name="const", bufs=1))
            res = ctx.enter_context(tc.tile_pool(name="res", bufs=1))
            psum = ctx.enter_context(
                tc.tile_pool(name="ps", bufs=1, space=bass.MemorySpace.PSUM)
            )
            outp = ctx.enter_context(tc.tile_pool(name="out", bufs=3))

            # bias[n] laid out [128, 16]: column t holds bias[t*128:(t+1)*128]
            bias_t = const.tile([P, NT], F32)
            nc.gpsimd.dma_start(
                out=bias_t[:], in_=bias.rearrange("(t p) -> p t", p=P)
            )

            def load_bin(dram, name, t, h, width, dma_eng):
                # DoubleRow operand chunk [128, 2, width] fp8, contiguous DMA.
                # Binarize in place: keep sign bit, force the rest to 1.0.
                # The pass runs on a uint16 view (two fp8 per ALU element).
                tl = res.tile([P, 2, width], FP8, tag=f"{name}{t}_{h}")
                dma_eng.dma_start(out=tl[:], in_=dram[t, h])
                nc.vector.tensor_scalar(
                    tl[:].bitcast(U16),
                    tl[:].bitcast(U16),
                    0x8080,
                    0x3838,
                    mybir.AluOpType.bitwise_and,
                    mybir.AluOpType.bitwise_or,
                )
                return tl

            # x: both halves feed every group -> interleave by strip.
            # w half 1 only feeds n-tiles 8..15 (groups 4+) -> defer all
            # of it behind half 0 so early HBM bandwidth unblocks group 0.
            wb = [[None, None] for _ in range(KT2)]
            xb = []
            for t in range(KT2):
                wb[t][0] = load_bin(wT, "w", t, 0, WCH, nc.sync)
                xb.append(
                    [
                        load_bin(xT, "x", t, 0, XCH, nc.scalar),
                        load_bin(xT, "x", t, 1, XCH, nc.scalar),
                    ]
                )
            for t in range(KT2):
                wb[t][1] = load_bin(wT, "w", t, 1, WCH, nc.sync)

            NPQ = WCH // P   # n-tiles per w chunk
            MCH = XCH // MC  # m-chunks per x half

            def w_slice(t, n):
                return wb[t][n // NPQ][
                    :, :, (n % NPQ) * P : (n % NPQ + 1) * P
                ]

            def x_slice(t, mc):
                return xb[t][mc // MCH][
                    :, :, (mc % MCH) * MC : (mc % MCH + 1) * MC
                ]

            NGRP = NT // NG
            for g in range(NGRP):
                pss = [
                    [
                        psum.tile(
                            [P, MC], F32, tag=f"ps{i}_{mc}", name=f"ps_{g}_{i}_{mc}"
                        )
                        for mc in range(MT)
                    ]
                    for i in range(NG)
                ]
                ots = [
                    outp.tile([P, MS], F32, tag=f"o{i}", name=f"o_{g}_{i}")
                    for i in range(NG)
                ]

                def evacuate(i, mc):
                    n = g * NG + i
                    nc.scalar.activation(
                        ots[i][:, mc * MC : (mc + 1) * MC],
                        pss[i][mc][:],
                        mybir.ActivationFunctionType.Identity,
                        bias=bias_t[:, n : n + 1],
                    )
                    nc.sync.dma_start(
                        out=outT[n * P : (n + 1) * P, mc * MC : (mc + 1) * MC],
                        in_=ots[i][:, mc * MC : (mc + 1) * MC],
                    )

                if g < NGRP - 1:
                    # k-tile outer: consume input strips as they stream in.
                    for t in range(KT2):
                        for i in range(NG):
                            for mc in range(MT):
                                nc.tensor.matmul(
                                    pss[i][mc][:],
                                    w_slice(t, g * NG + i),
                                    x_slice(t, mc),
                                    start=(t == 0),
                                    stop=(t == KT2 - 1),
                                    perf_mode=mybir.MatmulPerfMode.DoubleRow,
                                )
                    for i in range(NG):
                        for mc in range(MT):
                            evacuate(i, mc)
                else:
                    # Last group: bank-major so evacuation and output DMA of
                    # bank b overlap the matmuls of bank b+1 (shrinks the
                    # kernel tail to one bank's epilogue).
                    for i in range(NG):
                        for mc in range(MT):
                            for t in range(KT2):
                                nc.tensor.matmul(
                                    pss[i][mc][:],
                                    w_slice(t, g * NG + i),
                                    x_slice(t, mc),
                                    start=(t == 0),
                                    stop=(t == KT2 - 1),
                                    perf_mode=mybir.MatmulPerfMode.DoubleRow,
                                )
                            evacuate(i, mc)

    nc.compile()
    return nc


_NC = None


def _get_nc():
    global _NC
    if _NC is None:
        _NC = build_nc()
    return _NC


def _to_fp8_T(a, nchunks):
    # Transposed fp8 copy preserving the SIGN of every element exactly
    # (magnitudes are irrelevant downstream -- the device binarizes).
    # Magnitudes are clipped into e4m3 range so the cast can't flush to
    # zero or overflow, and exact zeros are encoded as tiny NEGATIVES
    # because reference binarize maps 0 -> -1.  The result is pre-tiled to
    # [t, chunk, p, j, c] so each device chunk is one contiguous DMA.
    at = a.T
    mag = np.clip(np.abs(at), 0.002, 240.0)
    enc = np.where(at > 0, mag, -mag).astype(ml_dtypes.float8_e4m3fn)
    kk, cols = enc.shape
    tiled = enc.reshape(
        kk // (2 * P), 2, P, nchunks, cols // nchunks
    ).transpose(0, 3, 2, 1, 4)
    return np.ascontiguousarray(tiled)


def make_in_maps(x, weight, bias):
    x = np.asarray(x, dtype=np.float32)
    weight = np.asarray(weight, dtype=np.float32)
    bias = np.asarray(bias, dtype=np.float32)
    wTb = _to_fp8_T(weight, 2)
    in_maps = []
    for i in range(NCORES):
        xTb = _to_fp8_T(x[i * MS : (i + 1) * MS, :], 2)
        in_maps.append({"xT": xTb, "wT": wTb, "bias": bias})
    return in_maps


def assemble_out(results):
    out = np.empty((MTOT, NF), dtype=np.float32)
    for i in range(NCORES):
        out[i * MS : (i + 1) * MS, :] = results[i]["outT"].T
    return out


def run(x, weight, bias, trace=False, **kwargs):
    nc = _get_nc()
    in_maps = make_in_maps(x, weight, bias)
    res = run_bass_kernel_spmd(
        nc, in_maps, list(range(NCORES)), trace=trace, **kwargs
    )
    return assemble_out(res.results), res


def kernel(x, weight, bias):
    out, _ = run(x, weight, bias)
    return out


# revision 20
# speedup vs baseline: 1.0159x; 1.0056x over previous
# BinarizeLinear on 8 Trainium2 NeuronCores.
#
# reference: out = binarize(x) @ binarize(weight).T + bias
#   x      [16384, 2048] f32
#   weight [2048, 2048]  f32
#   bias   [2048]        f32
#   out    [16384, 2048] f32
#
# Strategy (data-parallel over rows of x, weight/bias replicated):
#   - Each of the 8 cores gets a 2048-row shard of x.
#   - Host uploads x-shard and weight TRANSPOSED (K on the leading axis) so
#     the contraction dim lands on SBUF partitions with a natural contiguous
#     DMA.  Uploads are fp8e4m3 with magnitudes clipped into fp8 range and
#     zeros encoded as tiny negatives: a lossless SIGN encoding, which is the
#     only thing binarize consumes (reference maps 0 -> -1, hence -0 style
#     encoding for zeros).
#   - Device binarizes both operands to exactly +-1.0 with a single
#     tensor_scalar pass over a uint16 bitcast (two fp8 lanes per element):
#     (v & 0x8080) | 0x3838.
#   - out.T[n, m] = sum_k wbT[k, n] * xbT[k, m] accumulates in PSUM with
#     DoubleRow fp8 matmuls (2 MACs/cell/cycle, contraction 256 per MM).
#   - ScalarE evacuates PSUM with a fused per-partition bias add
#     (activation Identity, bias = bias[n] column), giving out.T + bias.
#   - Host transposes each core's out.T shard back and stacks.

import sys

import numpy as np

try:
    import concourse  # noqa: F401
except ImportError:
    sys.path.insert(0, "/opt/trn_rl_repo")

import ml_dtypes
from contextlib import ExitStack

import concourse.bass as bass
import concourse.mybir as mybir
import concourse.tile as tile
from concourse import bacc
from concourse.bass_utils import run_bass_kernel_spmd

NCORES = 8
K = 2048          # contraction dim (in_features)
NF = 2048         # out features
MTOT = 16384      # rows of x
MS = MTOT // NCORES  # rows per core
P = 128           # partitions
MC = 512          # moving free-dim chunk (one PSUM bank of f32)
KT2 = K // (2 * P)   # 8 double-k-tiles (DoubleRow contracts 256/MM)
NT = NF // P      # 16 n-tiles
MT = MS // MC     # 4 m-chunks

F32 = mybir.dt.float32
FP8 = mybir.dt.float8e4
U8 = mybir.dt.uint8


def build_nc(debug=False):
    nc = bacc.Bacc(
        "TRN2", target_bir_lowering=False, debug=debug, num_devices=NCORES
    )
    # Inputs arrive pre-tiled from the host: [t, h, p, j, c] =
    # transposed_tensor[(2t+j)*128 + p, h*CH + c], so each (t, h) chunk is
    # one fully contiguous [128, 2, CH] DMA (2KB per partition).
    XCH = NF // 2
    WCH = NF // 2
    xT = nc.dram_tensor(
        "xT", [K // (2 * P), 2, P, 2, XCH], FP8, kind="ExternalInput"
    ).ap()
    wT = nc.dram_tensor(
        "wT", [K // (2 * P), 2, P, 2, WCH], FP8, kind="ExternalInput"
    ).ap()
    bias = nc.dram_tensor("bias", [NF], F32, kind="ExternalInput").ap()
    outT = nc.dram_tensor("outT", [NF, MS], F32, kind="ExternalOutput").ap()

    U16 = mybir.dt.uint16
    NG = 2  # n-tiles per group; NG*MT psum banks live at once

    with tile.TileContext(nc) as tc:
        with ExitStack() as ctx:
            const = ctx.enter_context(tc.tile_pool(name="const", bufs=1))
            res = ctx.enter_context(tc.tile_pool(name="res", bufs=1))
            psum = ctx.enter_context(
                tc.tile_pool(name="ps", bufs=1, space=bass.MemorySpace.PSUM)
            )
            outp = ctx.enter_context(tc.tile_pool(name="out", bufs=3))

            # bias[n] laid out [128, 16]: column t holds bias[t*128:(t+1)*128]
            bias_t = const.tile([P, NT], F32)
            nc.gpsimd.dma_start(
                out=bias_t[:], in_=bias.rearrange("(t p) -> p t", p=P)
            )

            # PE warm-up: dummy DoubleRow matmuls on a zeroed tile fill the
            # dead window between the Tile preamble and the first real data,
            # so the HAM clock gate is at 2.4 GHz when real matmuls start.
            # They write a bank the first real group later reclaims with
            # start=True, so the garbage output is never observed.
            warm = const.tile([P, 2, MC], FP8, name="warm")
            nc.gpsimd.memset(warm[:], 0.0)
            warm_ps = psum.tile([P, MC], F32, tag="ps0_0", name="warm_ps")
            for wi in range(16):
                nc.tensor.matmul(
                    warm_ps[:],
                    warm[:, :, :P],
                    warm[:],
                    start=(wi == 0),
                    stop=(wi == 15),
                    perf_mode=mybir.MatmulPerfMode.DoubleRow,
                )

            def load_bin(dram, name, t, h, width, dma_eng):
                # DoubleRow operand chunk [128, 2, width] fp8, contiguous DMA.
                # Binarize in place: keep sign bit, force the rest to 1.0.
                # The pass runs on a uint16 view (two fp8 per ALU element).
                tl = res.tile([P, 2, width], FP8, tag=f"{name}{t}_{h}")
                dma_eng.dma_start(out=tl[:], in_=dram[t, h])
                nc.vector.tensor_scalar(
                    tl[:].bitcast(U16),
                    tl[:].bitcast(U16),
                    0x8080,
                    0x3838,
                    mybir.AluOpType.bitwise_and,
                    mybir.AluOpType.bitwise_or,
                )
                return tl

            # x: both halves feed every group -> interleave by strip.
            # w half 1 only feeds n-tiles 8..15 (groups 4+) -> defer all
            # of it behind half 0 so early HBM bandwidth unblocks group 0.
            wb = [[None, None] for _ in range(KT2)]
            xb = []
            for t in range(KT2):
                wb[t][0] = load_bin(wT, "w", t, 0, WCH, nc.sync)
                xb.append(
                    [
                        load_bin(xT, "x", t, 0, XCH, nc.scalar),
                        load_bin(xT, "x", t, 1, XCH, nc.scalar),
                    ]
                )
            for t in range(KT2):
                wb[t][1] = load_bin(wT, "w", t, 1, WCH, nc.sync)

            NPQ = WCH // P   # n-tiles per w chunk
            MCH = XCH // MC  # m-chunks per x half

            def w_slice(t, n):
                return wb[t][n // NPQ][
                    :, :, (n % NPQ) * P : (n % NPQ + 1) * P
                ]

            def x_slice(t, mc):
                return xb[t][mc // MCH][
                    :, :, (mc % MCH) * MC : (mc % MCH + 1) * MC
                ]

            NGRP = NT // NG
            for g in range(NGRP):
                pss = [
                    [
                        psum.tile(
                            [P, MC], F32, tag=f"ps{i}_{mc}", name=f"ps_{g}_{i}_{mc}"
                        )
                        for mc in range(MT)
                    ]
                    for i in range(NG)
                ]
                ots = [
                    outp.tile([P, MS], F32, tag=f"o{i}", name=f"o_{g}_{i}")
                    for i in range(NG)
                ]

                def evacuate(i, mc):
                    n = g * NG + i
                    nc.scalar.activation(
                        ots[i][:, mc * MC : (mc + 1) * MC],
                        pss[i][mc][:],
                        mybir.ActivationFunctionType.Identity,
                        bias=bias_t[:, n : n + 1],
                    )
                    # Outputs ride the scalar HWDGE queue: its FIFO already
                    # holds every x-input descriptor, so output traffic only
                    # reaches HBM after the input stream finishes and never
                    # steals ramp-phase read bandwidth.
                    nc.scalar.dma_start(
                        out=outT[n * P : (n + 1) * P, mc * MC : (mc + 1) * MC],
                        in_=ots[i][:, mc * MC : (mc + 1) * MC],
                    )

                if g < NGRP - 1:
                    # k-tile outer: consume input strips as they stream in.
                    for t in range(KT2):
                        for i in range(NG):
                            for mc in range(MT):
                                nc.tensor.matmul(
                                    pss[i][mc][:],
                                    w_slice(t, g * NG + i),
                                    x_slice(t, mc),
                                    start=(t == 0),
                                    stop=(t == KT2 - 1),
                                    perf_mode=mybir.MatmulPerfMode.DoubleRow,
                                )
                    for i in range(NG):
                        for mc in range(MT):
                            evacuate(i, mc)
                else:
                    # Last group: bank-major so evacuation and output DMA of
                    # bank b overlap the matmuls of bank b+1 (shrinks the
                    # kernel tail to one bank's epilogue).
                    for i in range(NG):
                        for mc in range(MT):
                            for t in range(KT2):
                                nc.tensor.matmul(
                                    pss[i][mc][:],
                                    w_slice(t, g * NG + i),
                                    x_slice(t, mc),
                                    start=(t == 0),
                                    stop=(t == KT2 - 1),
                                    perf_mode=mybir.MatmulPerfMode.DoubleRow,
                                )
                            evacuate(i, mc)

    nc.compile()
    return nc


_NC = None


def _get_nc():
    global _NC
    if _NC is None:
        _NC = build_nc()
    return _NC


def _to_fp8_T(a, nchunks):
    # Transposed fp8 copy preserving the SIGN of every element exactly
    # (magnitudes are irrelevant downstream -- the device binarizes).
    # Magnitudes are clipped into e4m3 range so the cast can't flush to
    # zero or overflow, and exact zeros are encoded as tiny NEGATIVES
    # because reference binarize maps 0 -> -1.  The result is pre-tiled to
    # [t, chunk, p, j, c] so each device chunk is one contiguous DMA.
    at = a.T
    mag = np.clip(np.abs(at), 0.002, 240.0)
    enc = np.where(at > 0, mag, -mag).astype(ml_dtypes.float8_e4m3fn)
    kk, cols = enc.shape
    tiled = enc.reshape(
        kk // (2 * P), 2, P, nchunks, cols // nchunks
    ).transpose(0, 3, 2, 1, 4)
    return np.ascontiguousarray(tiled)


def make_in_maps(x, weight, bias):
    x = np.asarray(x, dtype=np.float32)
    weight = np.asarray(weight, dtype=np.float32)
    bias = np.asarray(bias, dtype=np.float32)
    wTb = _to_fp8_T(weight, 2)
    in_maps = []
    for i in range(NCORES):
        xTb = _to_fp8_T(x[i * MS : (i + 1) * MS, :], 2)
        in_maps.append({"xT": xTb, "wT": wTb, "bias": bias})
    return in_maps


def assemble_out(results):
    out = np.empty((MTOT, NF), dtype=np.float32)
    for i in range(NCORES):
        out[i * MS : (i + 1) * MS, :] = results[i]["outT"].T
    return out


def run(x, weight, bias, trace=False, **kwargs):
    nc = _get_nc()
    in_maps = make_in_maps(x, weight, bias)
    res = run_bass_kernel_spmd(
        nc, in_maps, list(range(NCORES)), trace=trace, **kwargs
    )
    return assemble_out(res.results), res


def kernel(x, weight, bias):
    out, _ = run(x, weight, bias)
    return out
